# revision 26
# baseline (speedup 1.0000x reference)
"""Multi-head causal attention (B=2, T=2048, D=1024, H=16) on 8 trn2 NeuronCores.

Sharding: data-parallel over batch (2) x tensor-parallel over heads (4 groups of
4 heads). Core c handles batch c//4, head group c%4. Each core computes its
heads' attention and a partial output projection; the host sums the 4 partials
per batch and adds b_out.
"""

import sys

sys.path.insert(0, "/opt/trn_rl_repo")

import ml_dtypes
import numpy as np

import concourse.bass as bass
import concourse.tile as tile
from concourse import bacc, mybir
from concourse.bass_utils import run_bass_kernel_spmd

F32 = mybir.dt.float32
F32R = mybir.dt.float32r
BF16 = mybir.dt.bfloat16

B, T, D, H = 2, 2048, 1024, 16
DH = D // H            # 64
HG = 4                 # heads per core
GCOLS = HG * DH        # 256 columns of qkv per core
NKT = T // 128         # 16 k-tiles of 128
NQC = T // 512         # 4 q-chunks of 512
NDT = D // 128         # 8 d-tiles of 128 (contraction)

_CACHED = {}


def _build():
    nc = bacc.Bacc("TRN2", target_bir_lowering=False, debug=False, num_devices=8)

    xT = nc.dram_tensor("xT", [D, T], F32R, kind="ExternalInput").ap()
    wqkv = nc.dram_tensor("wqkv", [D, 3 * GCOLS], F32R, kind="ExternalInput").ap()
    bqp = nc.dram_tensor("bqp", [128, 4], F32, kind="ExternalInput").ap()
    bv = nc.dram_tensor("bv", [1, GCOLS], F32R, kind="ExternalInput").ap()
    wout = nc.dram_tensor("wout", [GCOLS, D], F32R, kind="ExternalInput").ap()
    mask = nc.dram_tensor("mask", [128, 128], F32, kind="ExternalInput").ap()
    # consts row 0: ones, row 1: sel head0 (cols 0-63), row 2: sel head1 (cols 64-127)
    consts = nc.dram_tensor("consts", [3, 128], F32R, kind="ExternalInput").ap()
    out = nc.dram_tensor("out", [T, D], F32, kind="ExternalOutput").ap()

    Exp = mybir.ActivationFunctionType.Exp

    with tile.TileContext(nc) as tc:
        with tc.tile_pool(name="const", bufs=1) as const, \
             tc.tile_pool(name="ps_qkv", bufs=2, space=bass.MemorySpace.PSUM) as ps_qkv, \
             tc.tile_pool(name="ps_s", bufs=3, space=bass.MemorySpace.PSUM) as ps_s, \
             tc.tile_pool(name="ps_o", bufs=3, space=bass.MemorySpace.PSUM) as ps_o, \
             tc.tile_pool(name="rpool", bufs=4) as rpool, \
             tc.tile_pool(name="ppool", bufs=5) as ppool, \
             tc.tile_pool(name="opool", bufs=2) as opool:

            # ---- small constants first (needed by earliest evictions) ----
            mask_sb = const.tile([128, 128], F32)
            nc.sync.dma_start(out=mask_sb, in_=mask[:, :])
            bqp_sb = const.tile([128, 4], F32)  # per-partition bias for QT/KT tiles
            nc.sync.dma_start(out=bqp_sb, in_=bqp[:, :])
            bv_sb = const.tile([1, GCOLS], F32R)
            nc.gpsimd.dma_start(out=bv_sb, in_=bv[:, :])
            ones1 = const.tile([1, 128], F32R)
            nc.sync.dma_start(out=ones1, in_=consts[0:1, :])
            sel0 = const.tile([1, 128], F32R)
            nc.sync.dma_start(out=sel0, in_=consts[1:2, :])
            sel1 = const.tile([1, 128], F32R)
            nc.sync.dma_start(out=sel1, in_=consts[2:3, :])

            # broadcast V-bias to all partitions: ones1.T @ bv  -> [128, GCOLS]
            bvb_ps = ps_s.tile([128, GCOLS], F32, tag="s", name="bvb")
            nc.tensor.matmul(bvb_ps, ones1, bv_sb, start=True, stop=True)
            bvb_sb = const.tile([128, GCOLS], F32)
            nc.vector.tensor_copy(bvb_sb, bvb_ps)

            # ---- chunked input loads, interleaved for early compute start ----
            # first chunk: weights on sync (HWDGE), xT t0 on gpsimd (SWDGE);
            # remaining xT as one large DMA per d-tile, alternating queues
            with tc.tile_pool(name="wpool", bufs=1) as wpool:
                xt_sb = wpool.tile([128, NDT, T], F32R)
                w_sb = wpool.tile([128, NDT, 3 * GCOLS], F32R)
                for a in range(NDT):
                    nc.sync.dma_start(
                        out=w_sb[:, a, :], in_=wqkv[a * 128 : (a + 1) * 128, :]
                    )
                    nc.gpsimd.dma_start(
                        out=xt_sb[:, a, 0:512], in_=xT[a * 128 : (a + 1) * 128, 0:512]
                    )
                for tch in range(1, NQC):
                    for a in range(NDT):
                        eng = nc.gpsimd if a % 2 == 0 else nc.sync
                        eng.dma_start(
                            out=xt_sb[:, a, tch * 512 : (tch + 1) * 512],
                            in_=xT[a * 128 : (a + 1) * 128, tch * 512 : (tch + 1) * 512],
                        )

                # QT/KT head-pair tiles [128 = 2 heads x 64 dh, T], V (+ones col)
                qt = [const.tile([128, T], BF16, name=f"qt{p}") for p in range(2)]
                kt = [const.tile([128, T], BF16, name=f"kt{p}") for p in range(2)]
                v_aug = const.tile([128, NKT, HG * 65], BF16)
                ones64 = const.tile([128, NKT * HG], F32)
                nc.vector.memset(ones64, 1.0)
                nc.vector.tensor_copy(
                    v_aug.rearrange("p k (h c) -> p (k h) c", c=65)[:, :, 64], ones64
                )
                # normalized attention-output^T per pair
                ot = [const.tile([128, T], F32R, name=f"ot{p}") for p in range(2)]
                wout_sb = const.tile([128, 2, D], F32R)
                for a in range(2):
                    nc.sync.dma_start(
                        out=wout_sb[:, a, :], in_=wout[a * 128 : (a + 1) * 128, :]
                    )

                # ---- per-chunk phases ----
                def qkv_chunk(qc):
                    qs = slice(qc * 512, (qc + 1) * 512)
                    # QT (jt 0,1) / KT (jt 2,3) for this chunk
                    for jt in range(4):
                        dst = qt[jt] if jt < 2 else kt[jt - 2]
                        ps = ps_qkv.tile([128, 512], F32, tag="qkv", name=f"qk_{jt}_{qc}")
                        for a in range(NDT):
                            nc.tensor.matmul(
                                ps,
                                w_sb[:, a, jt * 128 : (jt + 1) * 128],
                                xt_sb[:, a, qs],
                                start=(a == 0),
                                stop=(a == NDT - 1),
                            )
                        nc.vector.tensor_scalar_add(dst[:, qs], ps, bqp_sb[:, jt : jt + 1])
                    # V for this chunk's 4 k-tiles
                    for k in range(4 * qc, 4 * qc + 4):
                        ps = ps_qkv.tile([128, GCOLS], F32, tag="qkv", name=f"v_{k}")
                        for a in range(NDT):
                            nc.tensor.matmul(
                                ps,
                                xt_sb[:, a, k * 128 : (k + 1) * 128],
                                w_sb[:, a, 2 * GCOLS : 3 * GCOLS],
                                start=(a == 0),
                                stop=(a == NDT - 1),
                            )
                        for h in range(HG):
                            nc.vector.tensor_add(
                                v_aug[:, k, h * 65 : h * 65 + 64],
                                ps[:, h * DH : (h + 1) * DH],
                                bvb_sb[:, h * DH : (h + 1) * DH],
                            )

                def attention_chunk(qc):
                    qs = slice(qc * 512, (qc + 1) * 512)
                    for pr in range(2):
                        o_ps = [None, None]
                        for hh in range(2):
                            o_ps[hh] = ps_o.tile(
                                [65, 512], F32, tag="o", name=f"o_{qc}_{pr}_{hh}"
                            )
                        n_kt = 4 * qc + 4
                        for k in range(n_kt):
                            j = k - 4 * qc  # >= 0 on diagonal tiles
                            c0 = j * 128 if j >= 0 else 0
                            for hh in range(2):
                                half = slice(hh * 64, hh * 64 + 64)
                                s_ps = ps_s.tile(
                                    [128, 512], F32, tag="s", name=f"s_{qc}_{pr}_{k}_{hh}"
                                )
                                nc.tensor.matmul(
                                    s_ps[:, c0:512],
                                    kt[pr][half, k * 128 : (k + 1) * 128],
                                    qt[pr][half, qc * 512 + c0 : (qc + 1) * 512],
                                    start=True,
                                    stop=True,
                                )
                                if 0 <= j < 4:
                                    nc.vector.tensor_add(
                                        s_ps[:, j * 128 : (j + 1) * 128],
                                        s_ps[:, j * 128 : (j + 1) * 128],
                                        mask_sb,
                                    )
                                p_sb = ppool.tile([128, 512], BF16, tag="p")
                                nc.scalar.activation(
                                    p_sb[:, c0:512], s_ps[:, c0:512], Exp, scale=0.125
                                )
                                h_local = pr * 2 + hh
                                nc.tensor.matmul(
                                    o_ps[hh][:, c0:512],
                                    v_aug[:, k, h_local * 65 : (h_local + 1) * 65],
                                    p_sb[:, c0:512],
                                    start=(k == 0),
                                    stop=(k == n_kt - 1),
                                )
                        # denominators -> fast reciprocal -> broadcast to pair rows
                        r_sb = [None, None]
                        for hh in range(2):
                            rf = rpool.tile([1, 512], F32, tag="rf", name=f"rf_{qc}_{pr}_{hh}")
                            nc.vector.tensor_copy(rf, o_ps[hh][64:65, :])
                            nc.vector.reciprocal_approx_fast(out=rf, in_=rf)
                            r_sb[hh] = rpool.tile(
                                [1, 512], F32R, tag="r", name=f"r_{qc}_{pr}_{hh}"
                            )
                            nc.vector.tensor_copy(r_sb[hh], rf)
                        r2_ps = ps_s.tile([128, 512], F32, tag="s", name=f"r2_{qc}_{pr}")
                        nc.tensor.matmul(r2_ps, sel0, r_sb[0], start=True, stop=False)
                        nc.tensor.matmul(r2_ps, sel1, r_sb[1], start=False, stop=True)
                        r2_sb = rpool.tile([128, 512], F32, tag="r2sb", name=f"r2sb_{qc}_{pr}")
                        nc.vector.tensor_copy(r2_sb, r2_ps)
                        for hh in range(2):
                            half = slice(hh * 64, hh * 64 + 64)
                            nc.vector.tensor_mul(
                                ot[pr][half, qs], o_ps[hh][0:64, :], r2_sb[half, :]
                            )

                def proj_chunk(qc):
                    for tt in range(qc * 4, qc * 4 + 4):
                        o_sb = opool.tile([128, D], F32, tag="outsb")
                        for dc in range(2):
                            ps = ps_s.tile([128, 512], F32, tag="s", name=f"pr_{tt}_{dc}")
                            nc.tensor.matmul(
                                ps,
                                ot[0][:, tt * 128 : (tt + 1) * 128],
                                wout_sb[:, 0, dc * 512 : (dc + 1) * 512],
                                start=True,
                                stop=False,
                            )
                            nc.tensor.matmul(
                                ps,
                                ot[1][:, tt * 128 : (tt + 1) * 128],
                                wout_sb[:, 1, dc * 512 : (dc + 1) * 512],
                                start=False,
                                stop=True,
                            )
                            nc.vector.tensor_copy(o_sb[:, dc * 512 : (dc + 1) * 512], ps)
                        nc.gpsimd.dma_start(
                            out=out[tt * 128 : (tt + 1) * 128, :], in_=o_sb
                        )

                # main schedule: QKV(qc+1) emitted between attention(qc) and
                # proj(qc) so the PE has ready work while the normalization
                # critical path (sums->recip->r2) completes.
                for qc in range(NQC):
                    qkv_chunk(qc)
                    attention_chunk(qc)
                    proj_chunk(qc)

    nc.compile()
    return nc


def _mask_np():
    rk = np.arange(128)[:, None]
    cq = np.arange(128)[None, :]
    return np.where(rk <= cq, 0.0, -1.0e30).astype(np.float32)


def _consts_np():
    c = np.zeros((3, 128), dtype=np.float32)
    c[0, :] = 1.0
    c[1, 0:64] = 1.0
    c[2, 64:128] = 1.0
    return c


def _in_maps(x, w_qkv, b_qkv, w_out):
    mask = _mask_np()
    consts = _consts_np()
    xTs = [np.ascontiguousarray(x[b].T) for b in range(B)]
    in_maps = []
    for c in range(8):
        b, g = divmod(c, 4)
        cols = slice(g * GCOLS, (g + 1) * GCOLS)
        wq = np.concatenate(
            [w_qkv[:, cols], w_qkv[:, D:][:, cols], w_qkv[:, 2 * D :][:, cols]], axis=1
        )
        bq = np.concatenate([b_qkv[cols], b_qkv[D:][cols], b_qkv[2 * D :][cols]])
        in_maps.append(
            {
                "xT": xTs[b],
                "wqkv": np.ascontiguousarray(wq),
                "bqp": np.ascontiguousarray(bq[: 2 * GCOLS].reshape(4, 128).T),
                "bv": np.ascontiguousarray(bq[2 * GCOLS :].reshape(1, GCOLS)),
                "wout": np.ascontiguousarray(w_out[cols, :]),
                "mask": mask,
                "consts": consts,
            }
        )
    return in_maps


def kernel(x, w_qkv, b_qkv, w_out, b_out):
    x = np.ascontiguousarray(np.asarray(x, dtype=np.float32))
    w_qkv = np.ascontiguousarray(np.asarray(w_qkv, dtype=np.float32))
    b_qkv = np.asarray(b_qkv, dtype=np.float32)
    w_out = np.ascontiguousarray(np.asarray(w_out, dtype=np.float32))
    b_out = np.asarray(b_out, dtype=np.float32)

    if "nc" not in _CACHED:
        _CACHED["nc"] = _build()
    nc = _CACHED["nc"]

    res = run_bass_kernel_spmd(nc, _in_maps(x, w_qkv, b_qkv, w_out), list(range(8)))
    out = np.zeros((B, T, D), dtype=np.float32)
    for c in range(8):
        out[c // 4] += res.results[c]["out"]
    out += b_out
    return out
